# revision 62
# baseline (speedup 1.0000x reference)
"""Multi-head attention (B=4, S=2048, D=1024, H=16, dk=dv=64) on 8 TRN2 cores.

Sharding: core = (batch b, head-group g): data-parallel over batch (4) x
tensor-parallel over heads (2 groups of 8). Each core computes its batch's
Q/K/V projections for its 8 heads, attention, and a partial output
projection over its heads' rows of Wo. The host sums the two partial
outputs per batch.

All matmuls run in fp8 (e4m3) hi+lo residual pairs packed into
MatmulPerfMode.DoubleRow slots, keeping near-fp32 precision at fp8
DoubleRow rates (0.5 cycles/row):
  projections / out-proj: (x_hi+x_lo)@(w_hi+w_lo) minus the lo*lo term:
    3 term-passes, each DoubleRow matmul covering two 128-deep contraction
    chunks -> 12 (proj) / 6 (out) matmuls vs 8 / 4 at fp32r: 1.33x.
  scores: contract is only dk=64, so [K_hi;k_lo] ride the spare contract
    rows and the two DoubleRow i-slots carry [Q_hi| q_lo] broadcast: one
    matmul per (head, chunk) computes (Q_hi+q_lo)@(K_hi+k_lo): 2x, exact.
  AV + softmax stay fp32r (the exp output can't be hi/lo split without a
    full extra DVE pass over all S*S weights).

x and weights are hi/lo split on the host (weights prescaled by 32 so the
residuals stay in e4m3 normal range; descales fold into the exp scale and
the output staging copy). Q/K/hn are split on DVE and Pool (gpsimd) with
small SBUF->SBUF DMAs for the cross-partition duplicates.

The attention phase is ACT(exp)-bound (~1038ns per [128,1024] exp vs
~640ns of PE per chunk), so the PE stream is software-pipelined: the next
chunk's scores matmuls are emitted BEFORE the current chunk's AV matmuls
(PE queues are in-order; scores must run during the current exp so the
next exp starts back-to-back), and the remaining PE work (next pair's
projections, V chunks, trailing output projection) is drip-fed into the
exp-wait gaps via a pending-filler queue.
"""

import numpy as np
import ml_dtypes

import concourse.bacc as bacc
import concourse.tile as tile
import concourse.mybir as mybir
from concourse.bass_utils import run_bass_kernel_spmd

F32 = mybir.dt.float32
F32R = mybir.dt.float32r
F8 = mybir.dt.float8e4
E4 = ml_dtypes.float8_e4m3fn
EXP = mybir.ActivationFunctionType.Exp
DR = mybir.MatmulPerfMode.DoubleRow

P = 128
S = 2048
D = 1024
DK = 64
HPC = 8            # heads per core
NSC = S // P       # 16 s-chunks of 128
NST = 4            # s-tiles of 512
STW = S // NST     # 512
NDC = D // P       # 8 d_model chunks
NPAIR = HPC // 2   # 4 head pairs
NQB = S // STW     # 4 q-blocks of 512
WS = 32.0
ESCALE = (1.0 / np.sqrt(DK)) / (WS * WS)
OSCALE = 1.0 / (WS * WS)


def build_kernel():
    nc = bacc.Bacc("TRN2", target_bir_lowering=False, debug=False)

    xh_d = nc.dram_tensor("xh", [NST, P, NDC, STW], F8,
                          kind="ExternalInput")
    xl_d = nc.dram_tensor("xl", [NST, P, NDC, STW], F8,
                          kind="ExternalInput")
    wqh_d = nc.dram_tensor("wqh", [NPAIR, P, NDC, P], F8,
                           kind="ExternalInput")
    wql_d = nc.dram_tensor("wql", [NPAIR, P, NDC, P], F8,
                           kind="ExternalInput")
    wkh_d = nc.dram_tensor("wkh", [NPAIR, P, NDC, P], F8,
                           kind="ExternalInput")
    wkl_d = nc.dram_tensor("wkl", [NPAIR, P, NDC, P], F8,
                           kind="ExternalInput")
    wvh_d = nc.dram_tensor("wvh", [P, NDC, HPC * DK], F8,
                           kind="ExternalInput")
    wvl_d = nc.dram_tensor("wvl", [P, NDC, HPC * DK], F8,
                           kind="ExternalInput")
    woh_d = nc.dram_tensor("woh", [P, NPAIR, D], F8,
                           kind="ExternalInput")
    wol_d = nc.dram_tensor("wol", [P, NPAIR, D], F8,
                           kind="ExternalInput")
    ones_d = nc.dram_tensor("ones", [P, HPC], F32R, kind="ExternalInput")
    out_d = nc.dram_tensor("out", [S, D], F32, kind="ExternalOutput")

    with tile.TileContext(nc) as tc:
        with tc.tile_pool(name="persist", bufs=1) as persist, \
             tc.tile_pool(name="stage", bufs=6) as stage:
            # x resident, split per s-tile for fine-grained DMA overlap
            xh = [persist.tile([P, NDC, STW], F8, tag=f"xh{st}",
                               name=f"xh{st}") for st in range(NST)]
            xl = [persist.tile([P, NDC, STW], F8, tag=f"xl{st}",
                               name=f"xl{st}") for st in range(NST)]
            # V with ones column per head: [128 s, sc, head, 64+1], holds 32V
            v520 = persist.tile([P, NSC, HPC, DK + 1], F32R, tag="v520")
            # normalized heads hi/lo: [hv, ci, s]; ci = heads (2ci, 2ci+1)
            hnh = persist.tile([P, NPAIR, S], F8, tag="hnh")
            hnl = persist.tile([P, NPAIR, S], F8, tag="hnl")
            wvh = persist.tile([P, NDC, HPC * DK], F8, tag="wvh")
            wvl = persist.tile([P, NDC, HPC * DK], F8, tag="wvl")
            woh = persist.tile([P, NPAIR, D], F8, tag="woh")
            wol = persist.tile([P, NPAIR, D], F8, tag="wol")
            ones_sb = persist.tile([P, HPC], F32R, tag="ones_sb")

            # bulk loads ride the Activation hwdge queue so they never
            # block the SP queue's small latency-critical DMAs (scores
            # operand duplicates, hn writes)
            def dma_x(st):
                nc.scalar.dma_start(xh[st][:], xh_d.ap()[st])
                nc.scalar.dma_start(xl[st][:], xl_d.ap()[st])

            def dma_wv():
                nc.scalar.dma_start(wvh[:], wvh_d.ap())
                nc.scalar.dma_start(wvl[:], wvl_d.ap())
                nc.sync.dma_start(ones_sb[:], ones_d.ap())

            def dma_wo():
                nc.scalar.dma_start(woh[:], woh_d.ap())
                nc.scalar.dma_start(wol[:], wol_d.ap())

            with tc.tile_pool(name="wqkp", bufs=3) as wqkp, \
                 tc.tile_pool(name="qpool", bufs=3) as qpool, \
                 tc.tile_pool(name="kpool", bufs=3) as kpool, \
                 tc.tile_pool(name="scrp", bufs=2) as scrp, \
                 tc.tile_pool(name="expp", bufs=4) as expp, \
                 tc.tile_pool(name="smallp", bufs=2) as smallp, \
                 tc.tile_pool(name="bcsb", bufs=2) as bcsb, \
                 tc.tile_pool(name="qkps", bufs=2, space="PSUM") as qkps, \
                 tc.tile_pool(name="scps", bufs=2, space="PSUM") as scps, \
                 tc.tile_pool(name="avps", bufs=1, space="PSUM") as avps:

                def dr_chain(ps_ap, pairs):
                    n = len(pairs)
                    for i, (w, m) in enumerate(pairs):
                        nc.tensor.matmul(ps_ap, w, m,
                                         start=(i == 0), stop=(i == n - 1),
                                         perf_mode=DR)

                # ---------- weights ----------
                wtiles = {}

                def fetch_w(pr):
                    t = {}
                    for nm, d_ in (("qh", wqh_d), ("ql", wql_d),
                                   ("kh", wkh_d), ("kl", wkl_d)):
                        w = wqkp.tile([P, NDC, P], F8, tag=f"w{nm}",
                                      name=f"w{nm}")
                        nc.sync.dma_start(w[:], d_.ap()[pr])
                        t[nm] = w
                    wtiles[pr] = t

                # ---------- Q/K projection + scores-operand prep --------
                qk_tiles = {}

                def new_qk(pr):
                    t = (qpool.tile([P, S], F8, tag="qa", name="qa"),
                         qpool.tile([P, S], F8, tag="qb", name="qb"),
                         kpool.tile([P, 2, S], F8, tag="ka", name="ka"),
                         kpool.tile([P, 2, S], F8, tag="kb", name="kb"))
                    qk_tiles[pr] = t
                    return t

                def qk_proj_term(pr, st, which, term, ps_box,
                                 defer_prep=True, prep_eng=None,
                                 prep_hi_eng=None):
                    """One 4-matmul term pass of the Q or K projection for
                    s-tile st; term 0 allocates the psum, term 2 follows
                    with the hi/lo split prep (deferred to the filler
                    queue unless the scores that read it are emitted in
                    the same program region)."""
                    t = wtiles[pr]
                    wh = t["qh" if which == "q" else "kh"]
                    wl_ = t["ql" if which == "q" else "kl"]
                    if term == 0:
                        ps_box[0] = qkps.tile([P, STW], F32, tag="qkp", name="qkp")
                    ps = ps_box[0]
                    mov, sta = ((xh[st], wh), (xl[st], wh),
                                (xh[st], wl_))[term]
                    for i, dc in enumerate(range(0, NDC, 2)):
                        nc.tensor.matmul(
                            ps[:], sta[:, dc:dc + 2, :],
                            mov[:, dc:dc + 2, :],
                            start=(term == 0 and i == 0),
                            stop=(term == 2 and i == 3), perf_mode=DR)
                    if term == 2:
                        if defer_prep:
                            pending.insert(0, (150.0, lambda: qk_prep(
                                pr, st, ps, which), -1))
                        else:
                            qk_prep(pr, st, ps, which, eng=prep_eng,
                                    hi_eng=prep_hi_eng)

                def qk_prep(pr, st, ps, which, eng=None, hi_eng=None):
                    """Split the projection psum (heads A|B stacked on
                    partitions) into fp8 hi/lo scores operands.

                    Q ([P,S]): head A = [hi(0:64); lo(64:128)], head B =
                    [lo; hi]; the DoubleRow i-slots broadcast (stride 0).
                    K ([P,2,S]): slot0 = hi, slot1 = lo, duplicated across
                    both partition halves via SBUF->SBUF DMA.
                    """
                    eng = eng or nc.vector
                    hi_copy = (hi_eng.copy if hi_eng is not None
                               else (eng.tensor_copy if eng is not nc.vector
                                     else nc.vector.tensor_copy))
                    qa, qb_, ka, kb = qk_tiles[pr]
                    sl = slice(st * STW, (st + 1) * STW)
                    if which == "q":
                        sA = scrp.tile([P, STW], F8, tag="scrA", name="scrA")
                        hi_copy(qa[0:DK, sl], ps[0:DK, :])
                        eng.tensor_sub(sA[0:DK, :], ps[0:DK, :],
                                       qa[0:DK, sl])
                        nc.sync.dma_start(qa[DK:P, sl], sA[0:DK, :])
                        hi_copy(qb_[DK:P, sl], ps[DK:P, :])
                        eng.tensor_sub(sA[DK:P, :], ps[DK:P, :],
                                       qb_[DK:P, sl])
                        nc.sync.dma_start(qb_[0:DK, sl], sA[DK:P, :])
                    else:
                        hi_copy(ka[0:DK, 0, sl], ps[0:DK, :])
                        eng.tensor_sub(ka[0:DK, 1, sl], ps[0:DK, :],
                                       ka[0:DK, 0, sl])
                        nc.sync.dma_start(ka[DK:P, :, sl], ka[0:DK, :, sl])
                        hi_copy(kb[DK:P, 0, sl], ps[DK:P, :])
                        eng.tensor_sub(kb[DK:P, 1, sl], ps[DK:P, :],
                                       kb[DK:P, 0, sl])
                        nc.sync.dma_start(kb[0:DK, :, sl], kb[DK:P, :, sl])

                def push_qk_proj(pr, st):
                    """Queue the full QK projection of (pr, st) as filler
                    units (6 term passes; ~430ns each)."""
                    for which in ("q", "k"):
                        box = [None]
                        for term in range(3):
                            pending.append(
                                (430.0,
                                 lambda pr=pr, st=st, w=which, t=term,
                                 b=box: qk_proj_term(pr, st, w, t, b)))

                # ---------- V projection ----------
                def v_chunk(sc, hp):
                    """V projection for s-chunk sc, head pair hp (2 heads,
                    128 cols): 12 DoubleRow matmuls + v520 copies."""
                    st, scl = divmod(sc, 4)
                    c0 = hp * 2 * DK
                    ssl = slice(scl * P, (scl + 1) * P)
                    ps = qkps.tile([P, STW], F32, tag="qkp", name="vps")
                    prs = ([(xh[st][:, dc:dc + 2, ssl],
                             wvh[:, dc:dc + 2, c0:c0 + 2 * DK])
                            for dc in range(0, NDC, 2)] +
                           [(xh[st][:, dc:dc + 2, ssl],
                             wvl[:, dc:dc + 2, c0:c0 + 2 * DK])
                            for dc in range(0, NDC, 2)] +
                           [(xl[st][:, dc:dc + 2, ssl],
                             wvh[:, dc:dc + 2, c0:c0 + 2 * DK])
                            for dc in range(0, NDC, 2)])
                    dr_chain(ps[:, 0:2 * DK], prs)
                    nc.vector.tensor_copy(
                        v520[:, sc, 2 * hp:2 * hp + 2, 0:DK],
                        ps[:, 0:2 * DK].rearrange("p (h v) -> p h v", v=DK))
                    nc.gpsimd.tensor_copy(
                        v520[:, sc, 2 * hp:2 * hp + 2, DK:DK + 1],
                        ones_sb[:, 2 * hp:2 * hp + 2, None])

                # ---------- output projection ----------
                def out_group(sc, dmh, borrow_scps=False):
                    ssl = slice(sc * P, (sc + 1) * P)
                    dsl = slice(dmh * 512, (dmh + 1) * 512)
                    if borrow_scps:
                        # epilogue only: the scores psum banks are idle, so
                        # widen the drain rotation with them
                        psw = scps.tile([P, 2 * STW], F32, tag="scp",
                                        name="ops")
                        ps = psw[:, 0:D // 2]
                    else:
                        ps = qkps.tile([P, D // 2], F32, tag="qkp",
                                       name="ops")
                    prs = ([(hnh[:, ci:ci + 2, ssl], woh[:, ci:ci + 2, dsl])
                            for ci in range(0, NPAIR, 2)] +
                           [(hnh[:, ci:ci + 2, ssl], wol[:, ci:ci + 2, dsl])
                            for ci in range(0, NPAIR, 2)] +
                           [(hnl[:, ci:ci + 2, ssl], woh[:, ci:ci + 2, dsl])
                            for ci in range(0, NPAIR, 2)])
                    dr_chain(ps[:], prs)
                    osb = stage.tile([P, D // 2], F32, tag="ostage",
                                     name="osb")
                    nc.vector.tensor_scalar_mul(osb[:], ps[:],
                                                float(OSCALE))
                    nc.sync.dma_start(out_d.ap()[ssl, dsl], osb[:])

                # ---------- attention ----------
                pending = []
                slot_ctr = [0]

                def pop_fillers(budget=500.0):
                    # keep heavy filler units away from the 3 slots around
                    # each q-block boundary: their psum-buffer rotation
                    # couples the in-order PE stream to the DVE prep
                    # backlog exactly when the next block's scores must
                    # issue back-to-back
                    boundary = (slot_ctr[0] % NSC) in (15, 0)
                    popped = False
                    while pending:
                        ent = pending[0]
                        cost = ent[0]
                        if len(ent) > 2 and ent[2] > slot_ctr[0]:
                            break
                        if boundary and cost > 300.0:
                            break
                        if popped and cost > budget:
                            break
                        pending.pop(0)
                        ent[1]()
                        budget -= cost
                        popped = True
                        if budget <= 0:
                            break

                def flush_fillers():
                    while pending:
                        pending.pop(0)[1]()

                def scores(pr, qb, sc):
                    qa, qb_, ka, kb = qk_tiles[pr]
                    q0 = qb * STW
                    scp = scps.tile([P, 2 * STW], F32, tag="scp",
                                    name="scp")
                    for j, (kt, qt) in enumerate(((ka, qa), (kb, qb_))):
                        mov = qt[:, None, q0:q0 + STW].to_broadcast(
                            [P, 2, STW])
                        nc.tensor.matmul(
                            scp[:, j * STW:(j + 1) * STW],
                            kt[:, :, sc * P:(sc + 1) * P], mov,
                            start=True, stop=True, perf_mode=DR)
                    return scp

                def attn_norm(pr, qb, av, inline=False):
                    """Emit only the two psum->sbuf copies now (they free
                    the AV banks for the next q-block, so they must jump
                    the elementwise backlog: one on DVE, one on Pool);
                    defer the rest of the normalize chain to the filler
                    queue so it spreads over the next q-block's slots."""
                    q0 = qb * STW
                    avs2 = []
                    for j in range(2):
                        avs = bcsb.tile([DK + 1, STW], F32, tag=f"avs{j}",
                                        name="avs")
                        nc.vector.tensor_copy(avs[:], av[j][:])
                        avs2.append(avs)

                    def norm_rest(j):
                        teng = nc.vector if j == 0 else nc.gpsimd
                        avs = avs2[j]
                        rec = smallp.tile([1, STW], F32R, tag="rec",
                                          name="rec")
                        with nc.allow_low_precision(
                                reason="softmax recip feeds fp32r mul"):
                            nc.vector.reciprocal(rec[:], avs[DK:DK + 1, :])
                        bcs = bcsb.tile([DK, STW], F32R, tag="bcs",
                                        name="bcs")
                        nc.gpsimd.partition_broadcast(bcs[:], rec[:],
                                                      channels=DK)
                        tf = bcsb.tile([DK, STW], F32, tag="tf", name="tf")
                        teng.tensor_mul(tf[:], avs[0:DK, :], bcs[:])
                        sl = slice(q0, q0 + STW)
                        if j == 0:
                            nc.gpsimd.tensor_copy(hnh[0:DK, pr, sl], tf[:])
                            nc.vector.tensor_sub(hnl[0:DK, pr, sl], tf[:],
                                                 hnh[0:DK, pr, sl])
                        else:
                            hs = scrp.tile([DK, STW], F8, tag="hs",
                                           name="hs")
                            ls = scrp.tile([DK, STW], F8, tag="ls",
                                           name="ls")
                            nc.gpsimd.tensor_copy(hs[:], tf[:])
                            nc.vector.tensor_sub(ls[:], tf[:], hs[:])
                            nc.sync.dma_start(hnh[DK:P, pr, sl], hs[:])
                            nc.sync.dma_start(hnl[DK:P, pr, sl], ls[:])

                    def norm_chunk(j, c):
                        teng = nc.vector if j == 0 else nc.gpsimd
                        avs = avs2[j]
                        cs = slice(c * P, (c + 1) * P)
                        rec = smallp.tile([1, P], F32R, tag="recc",
                                          name="rec")
                        with nc.allow_low_precision(
                                reason="softmax recip feeds fp32r mul"):
                            nc.vector.reciprocal(rec[:],
                                                 avs[DK:DK + 1, cs])
                        bcs = bcsb.tile([DK, P], F32R, tag="bcsc",
                                        name="bcs")
                        nc.gpsimd.partition_broadcast(bcs[:], rec[:],
                                                      channels=DK)
                        tf = bcsb.tile([DK, P], F32, tag="tfc", name="tf")
                        teng.tensor_mul(tf[:], avs[0:DK, cs], bcs[:])
                        sl = slice(q0 + c * P, q0 + (c + 1) * P)
                        if j == 0:
                            nc.gpsimd.tensor_copy(hnh[0:DK, pr, sl],
                                                  tf[:])
                            nc.vector.tensor_sub(hnl[0:DK, pr, sl],
                                                 tf[:],
                                                 hnh[0:DK, pr, sl])
                        else:
                            hs = scrp.tile([DK, P], F8, tag="hsc",
                                           name="hs")
                            ls = scrp.tile([DK, P], F8, tag="lsc",
                                           name="ls")
                            nc.gpsimd.tensor_copy(hs[:], tf[:])
                            nc.vector.tensor_sub(ls[:], tf[:], hs[:])
                            nc.sync.dma_start(hnh[DK:P, pr, sl], hs[:])
                            nc.sync.dma_start(hnl[DK:P, pr, sl], ls[:])

                    if inline:
                        # final q-block: normalize in 128-wide chunks so
                        # the epilogue's output projection can start after
                        # the first chunk instead of the whole 512 chain
                        for c in range(4):
                            for j in range(2):
                                norm_chunk(j, c)
                    else:
                        for j in range(2):
                            pending.append((80.0,
                                            lambda j=j: norm_rest(j)))

                def new_av():
                    return [avps.tile([DK + 1, STW], F32, tag=f"av{j}",
                                      name=f"av{j}")
                            for j in range(2)]

                def attn_run(pr, slots, avm, qb_hook=None,
                             prelude=None, sched=None, lookahead=2):
                    """Software-pipelined attention over `slots` (list of
                    (qb, sc)). PE emission order per slot is
                    [AV(sc); scores(sc+2); fillers]: the AV blocks on
                    exp(sc) anyway, the scores stay one full exp ahead
                    (they execute right after AV), and fillers ride the
                    remaining exp-wait gap. scores depth 2 fits the
                    2-buffer scores psum because scp(sc+2) reuses
                    scp(sc)'s banks, freed exactly when AV(sc) unblocks.
                    """
                    scq = [scores(pr, *slots[0])]
                    if lookahead == 2 and len(slots) > 1:
                        scq.append(scores(pr, *slots[1]))
                    if prelude:
                        for fn in prelude:
                            fn()
                    for idx, (qb, sc) in enumerate(slots):
                        slot_ctr[0] += 1
                        scp = scq.pop(0)
                        ex = expp.tile([P, 2 * STW], F32R, tag="exp",
                                       name="ex")
                        nc.scalar.activation(ex[:], scp[:], EXP,
                                             scale=float(ESCALE))
                        if qb not in avm:
                            avm[qb] = new_av()
                        av = avm[qb]
                        for j in range(2):
                            nc.tensor.matmul(
                                av[j][:], v520[:, sc, 2 * pr + j, :],
                                ex[:, j * STW:(j + 1) * STW],
                                start=(sc == 0), stop=(sc == NSC - 1),
                                skip_group_check=True)
                        if sched and idx in sched:
                            for fn in sched[idx]:
                                fn()
                        if idx + lookahead < len(slots):
                            scq.append(scores(pr, *slots[idx + lookahead]))
                        if sc == NSC - 1:
                            attn_norm(pr, qb, av,
                                      inline=(idx == len(slots) - 1
                                              and pr == NPAIR - 1))
                            if qb_hook:
                                qb_hook(qb)
                        pop_fillers()

                # ================= main schedule =================
                fetch_w(0)
                dma_x(0)
                new_qk(0)

                # pair 0 prologue: per-st projection + V(heads 0,1) with
                # q-block-0 attention overlapped (shared avm across calls)
                avm0 = {}
                # st0 projection inline, then one 16-slot lookahead-1 run
                # for q-block 0 with a static per-slot emission schedule
                # for the st1-3 projections and V chunks
                for which in ("q", "k"):
                    box = [None]
                    for term in range(3):
                        qk_proj_term(0, 0, which, term, box,
                                     defer_prep=False, prep_eng=nc.vector)
                dma_wv()
                fetch_w(1)
                dma_x(1)

                def pterm(st, which, term, box):
                    return lambda: qk_proj_term(0, st, which, term, box,
                                                defer_prep=False,
                                                prep_eng=nc.vector)

                def vc(sc):
                    return lambda: v_chunk(sc, 0)

                b1q, b1k = [None], [None]
                b2q, b2k = [None], [None]
                b3q, b3k = [None], [None]
                sched = {
                    0: [vc(1), pterm(1, "q", 0, b1q), pterm(1, "q", 1, b1q)],
                    1: [vc(2), pterm(1, "q", 2, b1q), pterm(1, "k", 0, b1k)],
                    2: [vc(3), pterm(1, "k", 1, b1k), pterm(1, "k", 2, b1k)],
                    3: [vc(4), (lambda: dma_x(2))],
                    4: [vc(5), pterm(2, "q", 0, b2q)],
                    5: [vc(6), pterm(2, "q", 1, b2q), pterm(2, "q", 2, b2q)],
                    6: [vc(7), pterm(2, "k", 0, b2k), pterm(2, "k", 1, b2k)],
                    7: [pterm(2, "k", 2, b2k), vc(8), (lambda: dma_x(3))],
                    8: [vc(9), pterm(3, "q", 0, b3q)],
                    9: [vc(10), pterm(3, "q", 1, b3q), pterm(3, "q", 2, b3q)],
                    10: [vc(11), pterm(3, "k", 0, b3k), pterm(3, "k", 1, b3k)],
                    11: [pterm(3, "k", 2, b3k), vc(12)],
                    12: [vc(13)],
                    13: [vc(14)],
                    14: [vc(15)],
                }
                attn_run(0, [(0, sc) for sc in range(NSC)], avm0,
                         prelude=[vc(0)], sched=sched, lookahead=1)

                # pair 0, q-blocks 1-3: fillers = pair-1 QK proj + V(2,3)
                new_qk(1)
                for st in range(NST):
                    push_qk_proj(1, st)
                for sc in range(NSC):
                    pending.append((330.0, lambda sc=sc: v_chunk(sc, 1)))
                attn_run(0, [(qb, sc) for qb in range(1, NQB)
                             for sc in range(NSC)], avm0)
                flush_fillers()
                dma_wo()

                # pairs 1..3
                for pr in range(1, NPAIR):
                    if pr < NPAIR - 1:
                        fetch_w(pr + 1)
                        new_qk(pr + 1)
                        for st in range(NST):
                            push_qk_proj(pr + 1, st)
                        # V for the head pair consumed by pair pr+1
                        for sc in range(NSC):
                            pending.append(
                                (330.0,
                                 lambda sc=sc, hp=pr + 1: v_chunk(sc, hp)))
                        attn_run(pr, [(qb, sc) for qb in range(NQB)
                                      for sc in range(NSC)], {})
                        flush_fillers()
                    else:
                        # last pair: trail the output projection
                        def hook(qb):
                            if qb < NQB - 1:
                                base = slot_ctr[0]
                                units = [(sc, dmh)
                                         for sc in range(qb * 4,
                                                         (qb + 1) * 4)
                                         for dmh in range(2)]
                                for k, (sc, dmh) in enumerate(units):
                                    gate = base + 10 + (k * 6) // 7
                                    pending.append(
                                        (650.0,
                                         lambda sc=sc, dmh=dmh:
                                         out_group(sc, dmh), gate))

                        attn_run(pr, [(qb, sc) for qb in range(NQB)
                                      for sc in range(NSC)], {},
                                 qb_hook=hook)
                        flush_fillers()
                        for k, (sc, dmh) in enumerate(
                                (sc, dmh)
                                for sc in range((NQB - 1) * 4, NQB * 4)
                                for dmh in range(2)):
                            out_group(sc, dmh, borrow_scps=(k % 2 == 1))

    nc.compile()
    return nc


_NC_CACHE = None


def _get_nc():
    global _NC_CACHE
    if _NC_CACHE is None:
        _NC_CACHE = build_kernel()
    return _NC_CACHE


def _split8(a):
    hi = np.asarray(a, dtype=E4)
    lo = np.asarray(a - hi.astype(np.float32), dtype=E4)
    return hi, lo


def kernel(x, Wq, Wk, Wv, Wo):
    x = np.asarray(x, dtype=np.float32)
    Wq = np.asarray(Wq, dtype=np.float32)
    Wk = np.asarray(Wk, dtype=np.float32)
    Wv = np.asarray(Wv, dtype=np.float32)
    Wo = np.asarray(Wo, dtype=np.float32)
    B = x.shape[0]
    ones = np.ones((P, HPC), dtype=np.float32)

    in_maps = []
    for core in range(8):
        b, g = divmod(core, 2)
        hs = g * HPC
        xt = np.ascontiguousarray(x[b].T)
        xhi, xlo = _split8(xt)
        # device layouts: partition-major contiguous for cheap DMA
        xhi = np.ascontiguousarray(
            xhi.reshape(NDC, P, NST, STW).transpose(2, 1, 0, 3))
        xlo = np.ascontiguousarray(
            xlo.reshape(NDC, P, NST, STW).transpose(2, 1, 0, 3))
        wq = np.stack([
            np.concatenate([Wq[hs + 2 * p], Wq[hs + 2 * p + 1]], axis=1)
            for p in range(NPAIR)]) * WS
        wk = np.stack([
            np.concatenate([Wk[hs + 2 * p], Wk[hs + 2 * p + 1]], axis=1)
            for p in range(NPAIR)]) * WS
        wv = np.concatenate([Wv[hs + h] for h in range(HPC)], axis=1) * WS
        wo = np.ascontiguousarray(Wo[hs * DK:(hs + HPC) * DK, :]) * WS

        def wlay(w):  # [NPAIR, D, P] -> [NPAIR, P(part), NDC, P]
            return np.ascontiguousarray(
                w.reshape(NPAIR, NDC, P, P).transpose(0, 2, 1, 3))

        wqh, wql = _split8(wq)
        wkh, wkl = _split8(wk)
        wqh, wql = wlay(wqh), wlay(wql)
        wkh, wkl = wlay(wkh), wlay(wkl)
        wvh, wvl = _split8(wv)
        wvh = np.ascontiguousarray(
            wvh.reshape(NDC, P, HPC * DK).transpose(1, 0, 2))
        wvl = np.ascontiguousarray(
            wvl.reshape(NDC, P, HPC * DK).transpose(1, 0, 2))
        woh, wol = _split8(wo)
        woh = np.ascontiguousarray(
            woh.reshape(NPAIR, P, D).transpose(1, 0, 2))
        wol = np.ascontiguousarray(
            wol.reshape(NPAIR, P, D).transpose(1, 0, 2))
        in_maps.append({
            "xh": xhi, "xl": xlo,
            "wqh": wqh, "wql": wql, "wkh": wkh, "wkl": wkl,
            "wvh": wvh, "wvl": wvl, "woh": woh, "wol": wol,
            "ones": ones})

    nc = _get_nc()
    res = run_bass_kernel_spmd(nc, in_maps, core_ids=list(range(8))).results

    out = np.empty((B, S, D), dtype=np.float32)
    for b in range(B):
        out[b] = res[2 * b]["out"] + res[2 * b + 1]["out"]
    return out
